# revision 27
# baseline (speedup 1.0000x reference)
"""Causal self-attention Trainium2 kernel.

Problem: x [2, 2048, 1024], per-head stacked QKV weights Wkqv [16, 1024, 192]
(torch split order k,q,v), bias bkqv [16, 192]. Output [2, 2048, 1024]
(concat of per-head attention outputs, no output projection).

Sharding: tensor-parallel over heads — 8 cores x 2 heads each. No
collectives; host slices weights per core and concatenates outputs.

Per-core dataflow (all "transposed world": feature dim on partitions):
  - host passes xT [1024, 4096] (d-major; cols = batch0 seq ++ batch1 seq)
  - proj^T: psum[f_tile, n] = W[d, f_tile].T @ xT[d, n] accumulated over d
    -> kT/qT/vT [64*2heads, 2048] per batch (q pre-scaled by 1/8 on host)
  - scores^T tile: S^T[k_tile, q] = kT[:, k_tile].T @ qT[:, q_chunk]
  - additive causal mask on diagonal tiles, exp on ScalarE (no max
    subtraction: scores are O(1) bounded), result fp32r
  - out^T[65, q] += [V | 1].T @ expS^T accumulated over k tiles; row 64
    is the softmax denominator
  - PE-transpose out^T 128-col blocks -> [128, 65], divide by denom
    (per-partition scalar), DMA to out[b, q_block, head_cols]
"""

import os
import sys
import types

import numpy as np

B = 2
N = 2048
D = 1024
H = 16
HD = 64
NCORES = 8
HPC = H // NCORES  # heads per core = 2
FPC = 3 * HD * HPC  # 384 packed feature cols per core (k|k|q|q|v|v)
NCH = N // 512  # 4 q-chunks of 512 per batch
NKT = N // 128  # 16 k-tiles per batch
MASKVAL = -30000.0

LAST_EXEC_NS = None
_CACHE = {}


def _install_ntff_hook():
    try:
        import antenv.axon_hooks  # noqa: F401
        return
    except ImportError:
        pass
    try:
        import antenv
        from trn_agent_boot.trn_boot import _ntff_profile_via_ctypes
        hook = _ntff_profile_via_ctypes("/opt/axon/libaxon_pjrt.so")
        mod = types.ModuleType("antenv.axon_hooks")
        mod.get_axon_ntff_profile_hook = lambda: hook
        mod.set_axon_ntff_profile_hook = lambda h: None
        sys.modules["antenv.axon_hooks"] = mod
        antenv.axon_hooks = mod
    except Exception:
        pass


def _build():
    import concourse.bass as bass  # noqa: F401
    import concourse.mybir as mybir
    import concourse.tile as tile
    from concourse import bacc

    f32 = mybir.dt.float32
    f32r = mybir.dt.float32r
    Exp = mybir.ActivationFunctionType.Exp

    nc = bacc.Bacc("TRN2", target_bir_lowering=False, debug=False,
                   num_devices=NCORES)

    xT_d = nc.dram_tensor("xT", [D, B * N], f32r, kind="ExternalInput").ap()
    W_d = nc.dram_tensor("W", [D, FPC], f32r, kind="ExternalInput").ap()
    b_d = nc.dram_tensor("bias", [FPC], f32, kind="ExternalInput").ap()
    m_d = nc.dram_tensor("mask", [128, 384], f32, kind="ExternalInput").ap()
    c_d = nc.dram_tensor("consts", [128, 192], f32r,
                         kind="ExternalInput").ap()
    o_d = nc.dram_tensor("out", [B, N, HPC * HD], f32,
                         kind="ExternalOutput").ap()

    with tile.TileContext(nc) as tc:
        with (
            tc.tile_pool(name="singles", bufs=1) as singles,
            tc.tile_pool(name="xt", bufs=16) as xt_pool,
            tc.tile_pool(name="kq", bufs=4) as kq_pool,
            tc.tile_pool(name="vt", bufs=1) as vt_pool,
            tc.tile_pool(name="vp", bufs=2) as vp_pool,
            tc.tile_pool(name="est", bufs=10) as est_pool,
            tc.tile_pool(name="ot", bufs=4) as ot_pool,
            tc.tile_pool(name="of", bufs=4) as of_pool,
            tc.tile_pool(name="ps_st", bufs=6, space="PSUM") as ps_st,
            tc.tile_pool(name="ps_ov", bufs=2, space="PSUM") as ps_ov,
        ):
            consts = singles.tile([128, 192], f32r)
            nc.gpsimd.dma_start(out=consts, in_=c_d)
            ident = consts[:, 0:128]
            mask_sb = singles.tile([128, 384], f32)
            nc.gpsimd.dma_start(out=mask_sb, in_=m_d)
            bias_sb = singles.tile([128, 3], f32)
            nc.gpsimd.dma_start(out=bias_sb,
                                in_=b_d.rearrange("(t p) -> p t", p=128))
            W_sb = singles.tile([128, 8, FPC], f32r)
            nc.sync.dma_start(out=W_sb,
                              in_=W_d.rearrange("(t p) f -> p t f", p=128))

            for b in range(B):
                # ---- load xT (per d-tile, per 512-col chunk) and project
                # kT/qT/vT [128, N] (2 heads stacked), chunk-outer so matmuls
                # start as soon as the first chunk of columns lands ----
                kT = kq_pool.tile([128, N], f32r, tag="kq")
                qT = kq_pool.tile([128, N], f32r, tag="kq")
                vT = vt_pool.tile([128, N], f32r, tag="vt")
                dest = [kT, qT, vT]
                xts = [[None] * 8 for _ in range(NCH)]

                def load_chunk(ch):
                    for d in range(8):
                        t = xt_pool.tile([128, 512], f32r, tag="xt",
                                         name=f"xt_{b}_{ch}_{d}")
                        nc.sync.dma_start(
                            out=t,
                            in_=xT_d[128 * d:128 * (d + 1),
                                     N * b + 512 * ch:N * b + 512 * (ch + 1)])
                        xts[ch][d] = t

                load_chunk(0)
                for ch in range(NCH):
                    for fi in range(3):
                        pj = ps_st.tile([128, 512], f32, tag="st")
                        for d in range(8):
                            nc.tensor.matmul(
                                pj,
                                W_sb[:, d, 128 * fi:128 * (fi + 1)],
                                xts[ch][d],
                                start=(d == 0), stop=(d == 7))
                        nc.vector.tensor_scalar_add(
                            out=dest[fi][:, 512 * ch:512 * (ch + 1)],
                            in0=pj, scalar1=bias_sb[:, fi:fi + 1])
                    if ch + 1 < NCH:
                        load_chunk(ch + 1)

                # ---- V natural layout [k, hd] with ones column ----
                vp_b = vp_pool.tile([128, HPC, NKT, 66], f32r, tag="vp")
                nc.sync.dma_start(
                    out=vp_b[:, :, :, 64:66],
                    in_=c_d[:, 128:192].rearrange(
                        "p (h t o) -> p h t o", h=HPC, o=2))
                for h in range(HPC):
                    for t in range(NKT):
                        vtp = ps_st.tile([128, 66], f32r, tag="st")
                        nc.tensor.transpose(
                            out=vtp[:, 0:64],
                            in_=vT[64 * h:64 * (h + 1),
                                   128 * t:128 * (t + 1)],
                            identity=ident[64 * h:64 * (h + 1),
                                           64 * h:64 * (h + 1)])
                        nc.vector.tensor_copy(out=vp_b[:, h, t, 0:64],
                                              in_=vtp[:, 0:64])

                # ---- attention: both heads' chains interleaved per
                # q-chunk so PE always has independent work while the
                # scores->mask->exp chain of the other head is in flight ----
                def ov_copy(ov, h, ch):
                    otsb = ot_pool.tile([66, 512], f32r, tag="ot",
                                        name=f"otsb_{b}_{ch}_{h}")
                    nc.vector.tensor_copy(out=otsb, in_=ov)
                    return otsb

                def normalize_emit(otsb, h, ch):
                    for s in range(4):
                        ott = ps_st.tile([128, 66], f32r, tag="st")
                        nc.tensor.transpose(
                            out=ott,
                            in_=otsb[:, 128 * s:128 * (s + 1)],
                            identity=ident[0:66, 0:66])
                        rec = of_pool.tile([128, 1], f32, tag="rec")
                        nc.vector.reciprocal(out=rec, in_=ott[:, 64:65])
                        of = of_pool.tile([128, 64], f32, tag="of")
                        nc.vector.tensor_scalar_mul(
                            out=of, in0=ott[:, 0:64], scalar1=rec)
                        nc.gpsimd.dma_start(
                            out=o_d[b,
                                    512 * ch + 128 * s:
                                    512 * ch + 128 * (s + 1),
                                    64 * h:64 * (h + 1)],
                            in_=of)

                def params(t, ch):
                    r = t - 4 * ch
                    if r < 0:
                        c0 = 0
                    elif r <= 2:
                        c0 = 128 * r
                    else:
                        c0 = 256
                    return r, c0, 512 - c0

                pending = []
                for ch in range(NCH):
                    nkt = 4 * ch + 4
                    ovs = [ps_ov.tile([66, 512], f32, tag="ov",
                                      name=f"ov_{b}_{ch}_{i}")
                           for i in range(HPC)]
                    ests = [{} for _ in range(HPC)]

                    def stage_scores(h, t, ch=ch, ests=ests):
                        r, c0, w = params(t, ch)
                        hs = slice(64 * h, 64 * (h + 1))
                        st = ps_st.tile([128, 512], f32, tag="st")
                        nc.tensor.matmul(
                            st[:, 0:w],
                            kT[hs, 128 * t:128 * (t + 1)],
                            qT[hs, 512 * ch + c0:512 * (ch + 1)],
                            start=True, stop=True,
                            tile_position=(64 * h, 0))
                        if r >= 0:
                            if r <= 2:
                                nc.vector.tensor_add(
                                    out=st[:, 0:128], in0=st[:, 0:128],
                                    in1=mask_sb[:, 0:128])
                            else:
                                nc.vector.tensor_add(
                                    out=st[:, 0:256], in0=st[:, 0:256],
                                    in1=mask_sb[:, 128:384])
                        est = est_pool.tile([128, 512], f32r, tag="est")
                        nc.scalar.activation(out=est[:, 0:w],
                                             in_=st[:, 0:w], func=Exp)
                        ests[h][t] = est

                    def stage_av(h, t, ch=ch, nkt=nkt, ests=ests, ovs=ovs):
                        r, c0, w = params(t, ch)
                        nc.tensor.matmul(
                            ovs[h][:, c0:512],
                            vp_b[:, h, t, :],
                            ests[h].pop(t)[:, 0:w],
                            start=(t == 0), stop=(t == nkt - 1))

                    LOOK = 2
                    for t in range(min(LOOK, nkt)):
                        for h in range(HPC):
                            stage_scores(h, t)
                    for otsb_p, h_p, ch_p in pending:
                        normalize_emit(otsb_p, h_p, ch_p)
                    pending = []
                    for t in range(nkt):
                        if t + LOOK < nkt:
                            for h in range(HPC):
                                stage_scores(h, t + LOOK)
                        for h in range(HPC):
                            stage_av(h, t)
                    for h in range(HPC):
                        pending.append((ov_copy(ovs[h], h, ch), h, ch))
                for otsb_p, h_p, ch_p in pending:
                    normalize_emit(otsb_p, h_p, ch_p)

    nc.compile()
    return nc


def _host_prep(x, Wkqv, bkqv):
    xT = np.ascontiguousarray(
        np.concatenate([x[0].T, x[1].T], axis=1)).astype(np.float32)

    # causal additive masks (k = partition/row, q = col), packed [128, 384]:
    # cols 0:128  -> triangle block: 0 where k <= q else MASKVAL
    # cols 128:384 -> shifted block: 0 where k <= q - 128 else MASKVAL
    kk = np.arange(128)[:, None]
    mt = np.where(kk <= np.arange(128)[None, :], 0.0, MASKVAL)
    md = np.where(kk <= np.arange(256)[None, :] - 128, 0.0, MASKVAL)
    mask = np.concatenate([mt, md], axis=1).astype(np.float32)

    consts = np.concatenate(
        [np.eye(128, dtype=np.float32),
         np.ones((128, 64), dtype=np.float32)], axis=1)

    in_maps = []
    for c in range(NCORES):
        Wc = np.empty((D, FPC), dtype=np.float32)
        bc = np.empty((FPC,), dtype=np.float32)
        for i, h in enumerate((HPC * c, HPC * c + 1)):
            Wc[:, 64 * i:64 * (i + 1)] = Wkqv[h][:, 0:64]          # k
            Wc[:, 128 + 64 * i:128 + 64 * (i + 1)] = \
                Wkqv[h][:, 64:128] / 8.0                            # q scaled
            Wc[:, 256 + 64 * i:256 + 64 * (i + 1)] = Wkqv[h][:, 128:192]  # v
            bc[64 * i:64 * (i + 1)] = bkqv[h][0:64]
            bc[128 + 64 * i:128 + 64 * (i + 1)] = bkqv[h][64:128] / 8.0
            bc[256 + 64 * i:256 + 64 * (i + 1)] = bkqv[h][128:192]
        in_maps.append({"xT": xT, "W": np.ascontiguousarray(Wc),
                        "bias": bc, "mask": mask, "consts": consts})
    return in_maps


def kernel(x, Wkqv, bkqv):
    global LAST_EXEC_NS
    _install_ntff_hook()
    from concourse.bass_utils import run_bass_kernel_spmd

    if "nc" not in _CACHE:
        _CACHE["nc"] = _build()
    nc = _CACHE["nc"]

    x = np.asarray(x, dtype=np.float32)
    Wkqv = np.asarray(Wkqv, dtype=np.float32)
    bkqv = np.asarray(bkqv, dtype=np.float32)
    in_maps = _host_prep(x, Wkqv, bkqv)

    trace = os.environ.get("BASS_KERNEL_TRACE", "0") == "1"
    res = run_bass_kernel_spmd(nc, in_maps, list(range(NCORES)), trace=trace)
    LAST_EXEC_NS = res.exec_time_ns

    out = np.empty((B, N, D), dtype=np.float32)
    for c in range(NCORES):
        out[:, :, 128 * c:128 * (c + 1)] = res.results[c]["out"]
    return out


# revision 29
# speedup vs baseline: 1.3156x; 1.3156x over previous
"""Causal self-attention Trainium2 kernel.

Problem: x [2, 2048, 1024], per-head stacked QKV weights Wkqv [16, 1024, 192]
(torch split order k,q,v), bias bkqv [16, 192]. Output [2, 2048, 1024]
(concat of per-head attention outputs, no output projection).

Sharding: tensor-parallel over heads — 8 cores x 2 heads each. No
collectives; host slices weights per core and concatenates outputs.

Per-core dataflow (all "transposed world": feature dim on partitions):
  - host passes xT [1024, 4096] (d-major; cols = batch0 seq ++ batch1 seq)
  - proj^T: psum[f_tile, n] = W[d, f_tile].T @ xT[d, n] accumulated over d
    -> kT/qT/vT [64*2heads, 2048] per batch (q pre-scaled by 1/8 on host)
  - scores^T tile: S^T[k_tile, q] = kT[:, k_tile].T @ qT[:, q_chunk]
  - additive causal mask on diagonal tiles, exp on ScalarE (no max
    subtraction: scores are O(1) bounded), result fp32r
  - out^T[65, q] += [V | 1].T @ expS^T accumulated over k tiles; row 64
    is the softmax denominator
  - PE-transpose out^T 128-col blocks -> [128, 65], divide by denom
    (per-partition scalar), DMA to out[b, q_block, head_cols]
"""

import os
import sys
import types

import numpy as np

B = 2
N = 2048
D = 1024
H = 16
HD = 64
NCORES = 8
HPC = H // NCORES  # heads per core = 2
FPC = 3 * HD * HPC  # 384 packed feature cols per core (k|k|q|q|v|v)
NCH = N // 512  # 4 q-chunks of 512 per batch
NKT = N // 128  # 16 k-tiles per batch
MASKVAL = -30000.0

LAST_EXEC_NS = None
_CACHE = {}


def _install_ntff_hook():
    try:
        import antenv.axon_hooks  # noqa: F401
        return
    except ImportError:
        pass
    try:
        import antenv
        from trn_agent_boot.trn_boot import _ntff_profile_via_ctypes
        hook = _ntff_profile_via_ctypes("/opt/axon/libaxon_pjrt.so")
        mod = types.ModuleType("antenv.axon_hooks")
        mod.get_axon_ntff_profile_hook = lambda: hook
        mod.set_axon_ntff_profile_hook = lambda h: None
        sys.modules["antenv.axon_hooks"] = mod
        antenv.axon_hooks = mod
    except Exception:
        pass


def _build():
    import concourse.bass as bass  # noqa: F401
    import concourse.mybir as mybir
    import concourse.tile as tile
    from concourse import bacc

    f32 = mybir.dt.float32
    f32r = mybir.dt.float32r
    bf16 = mybir.dt.bfloat16
    Exp = mybir.ActivationFunctionType.Exp

    nc = bacc.Bacc("TRN2", target_bir_lowering=False, debug=False,
                   num_devices=NCORES)

    xT_d = nc.dram_tensor("xT", [D, B * N], f32r, kind="ExternalInput").ap()
    W_d = nc.dram_tensor("W", [D, FPC], f32r, kind="ExternalInput").ap()
    b_d = nc.dram_tensor("bias", [FPC], f32, kind="ExternalInput").ap()
    m_d = nc.dram_tensor("mask", [128, 384], f32, kind="ExternalInput").ap()
    c_d = nc.dram_tensor("consts", [128, 192], f32r,
                         kind="ExternalInput").ap()
    cb_d = nc.dram_tensor("conesb", [128, 64], bf16,
                          kind="ExternalInput").ap()
    o_d = nc.dram_tensor("out", [B, N, HPC * HD], f32,
                         kind="ExternalOutput").ap()

    with tile.TileContext(nc) as tc:
        with (
            tc.tile_pool(name="singles", bufs=1) as singles,
            tc.tile_pool(name="xt", bufs=16) as xt_pool,
            tc.tile_pool(name="kq", bufs=4) as kq_pool,
            tc.tile_pool(name="vt", bufs=1) as vt_pool,
            tc.tile_pool(name="vp", bufs=2) as vp_pool,
            tc.tile_pool(name="est", bufs=10) as est_pool,
            tc.tile_pool(name="ot", bufs=4) as ot_pool,
            tc.tile_pool(name="of", bufs=4) as of_pool,
            tc.tile_pool(name="ps_st", bufs=5, space="PSUM") as ps_st,
            tc.tile_pool(name="ps_ov", bufs=3, space="PSUM") as ps_ov,
        ):
            consts = singles.tile([128, 192], f32r)
            nc.gpsimd.dma_start(out=consts, in_=c_d)
            ident = consts[:, 0:128]
            mask_sb = singles.tile([128, 384], f32)
            nc.gpsimd.dma_start(out=mask_sb, in_=m_d)
            bias_sb = singles.tile([128, 3], f32)
            nc.gpsimd.dma_start(out=bias_sb,
                                in_=b_d.rearrange("(t p) -> p t", p=128))
            W_sb = singles.tile([128, 8, FPC], f32r)
            nc.sync.dma_start(out=W_sb,
                              in_=W_d.rearrange("(t p) f -> p t f", p=128))

            for b in range(B):
                # ---- load xT (per d-tile, per 512-col chunk) and project
                # kT/qT/vT [128, N] (2 heads stacked), chunk-outer so matmuls
                # start as soon as the first chunk of columns lands ----
                kT = kq_pool.tile([128, N], f32r, tag="kq")
                qT = kq_pool.tile([128, N], f32r, tag="kq")
                vT = vt_pool.tile([128, N], f32r, tag="vt")
                dest = [kT, qT, vT]
                xts = [[None] * 8 for _ in range(NCH)]

                def load_chunk(ch):
                    for d in range(8):
                        t = xt_pool.tile([128, 512], f32r, tag="xt",
                                         name=f"xt_{b}_{ch}_{d}")
                        nc.sync.dma_start(
                            out=t,
                            in_=xT_d[128 * d:128 * (d + 1),
                                     N * b + 512 * ch:N * b + 512 * (ch + 1)])
                        xts[ch][d] = t

                load_chunk(0)
                for ch in range(NCH):
                    for fi in range(3):
                        pj = ps_st.tile([128, 512], f32, tag="st")
                        for d in range(8):
                            nc.tensor.matmul(
                                pj,
                                W_sb[:, d, 128 * fi:128 * (fi + 1)],
                                xts[ch][d],
                                start=(d == 0), stop=(d == 7))
                        nc.vector.tensor_scalar_add(
                            out=dest[fi][:, 512 * ch:512 * (ch + 1)],
                            in0=pj, scalar1=bias_sb[:, fi:fi + 1])
                    if ch + 1 < NCH:
                        load_chunk(ch + 1)

                # ---- V natural layout [k, hd] with ones column ----
                vp_b = vp_pool.tile([128, HPC, NKT, 66], bf16, tag="vp")
                nc.sync.dma_start(
                    out=vp_b[:, :, :, 64:66],
                    in_=cb_d.rearrange(
                        "p (h t o) -> p h t o", h=HPC, o=2))
                for h in range(HPC):
                    for t in range(NKT):
                        vtp = ps_ov.tile([128, 66], f32r, tag="ov")
                        nc.tensor.transpose(
                            out=vtp[:, 0:64],
                            in_=vT[64 * h:64 * (h + 1),
                                   128 * t:128 * (t + 1)],
                            identity=ident[64 * h:64 * (h + 1),
                                           64 * h:64 * (h + 1)])
                        nc.vector.tensor_copy(out=vp_b[:, h, t, 0:64],
                                              in_=vtp[:, 0:64])

                # ---- attention: both heads' chains interleaved per
                # q-chunk so PE always has independent work while the
                # scores->mask->exp chain of the other head is in flight ----
                def ov_copy(ov, h, ch):
                    otsb = ot_pool.tile([66, 512], f32r, tag="ot",
                                        name=f"otsb_{b}_{ch}_{h}")
                    nc.vector.tensor_copy(out=otsb, in_=ov)
                    return otsb

                def normalize_emit(otsb, h, ch):
                    for s in range(4):
                        ott = ps_ov.tile([128, 66], f32r, tag="ov")
                        nc.tensor.transpose(
                            out=ott,
                            in_=otsb[:, 128 * s:128 * (s + 1)],
                            identity=ident[0:66, 0:66])
                        rec = of_pool.tile([128, 1], f32, tag="rec")
                        nc.vector.reciprocal(out=rec, in_=ott[:, 64:65])
                        of = of_pool.tile([128, 64], f32, tag="of")
                        nc.vector.tensor_scalar_mul(
                            out=of, in0=ott[:, 0:64], scalar1=rec)
                        nc.gpsimd.dma_start(
                            out=o_d[b,
                                    512 * ch + 128 * s:
                                    512 * ch + 128 * (s + 1),
                                    64 * h:64 * (h + 1)],
                            in_=of)

                def params(t, ch):
                    r = t - 4 * ch
                    if r < 0:
                        c0 = 0
                    elif r <= 2:
                        c0 = 128 * r
                    else:
                        c0 = 256
                    return r, c0, 512 - c0

                pending = []
                for ch in range(NCH):
                    nkt = 4 * ch + 4
                    ovs = [ps_ov.tile([66, 512], f32, tag="ov",
                                      name=f"ov_{b}_{ch}_{i}")
                           for i in range(HPC)]
                    ests = [{} for _ in range(HPC)]

                    def stage_scores(h, t, ch=ch, ests=ests):
                        r, c0, w = params(t, ch)
                        hs = slice(64 * h, 64 * (h + 1))
                        st = ps_st.tile([128, 512], f32, tag="st")
                        nc.tensor.matmul(
                            st[:, 0:w],
                            kT[hs, 128 * t:128 * (t + 1)],
                            qT[hs, 512 * ch + c0:512 * (ch + 1)],
                            start=True, stop=True,
                            tile_position=(64 * h, 0))
                        if r >= 0:
                            if r <= 2:
                                nc.vector.tensor_add(
                                    out=st[:, 0:128], in0=st[:, 0:128],
                                    in1=mask_sb[:, 0:128])
                            else:
                                nc.vector.tensor_add(
                                    out=st[:, 0:256], in0=st[:, 0:256],
                                    in1=mask_sb[:, 128:384])
                        est = est_pool.tile([128, 512], bf16, tag="est")
                        nc.scalar.activation(out=est[:, 0:w],
                                             in_=st[:, 0:w], func=Exp)
                        ests[h][t] = est

                    def stage_av(h, t, ch=ch, nkt=nkt, ests=ests, ovs=ovs):
                        r, c0, w = params(t, ch)
                        nc.tensor.matmul(
                            ovs[h][:, c0:512],
                            vp_b[:, h, t, :],
                            ests[h].pop(t)[:, 0:w],
                            start=(t == 0), stop=(t == nkt - 1))

                    LOOK = 2
                    for t in range(min(LOOK, nkt)):
                        for h in range(HPC):
                            stage_scores(h, t)
                    for otsb_p, h_p, ch_p in pending:
                        normalize_emit(otsb_p, h_p, ch_p)
                    pending = []
                    for t in range(nkt):
                        if t + LOOK < nkt:
                            for h in range(HPC):
                                stage_scores(h, t + LOOK)
                        for h in range(HPC):
                            stage_av(h, t)
                    for h in range(HPC):
                        pending.append((ov_copy(ovs[h], h, ch), h, ch))
                for otsb_p, h_p, ch_p in pending:
                    normalize_emit(otsb_p, h_p, ch_p)

    nc.compile()
    return nc


def _host_prep(x, Wkqv, bkqv):
    xT = np.ascontiguousarray(
        np.concatenate([x[0].T, x[1].T], axis=1)).astype(np.float32)

    # causal additive masks (k = partition/row, q = col), packed [128, 384]:
    # cols 0:128  -> triangle block: 0 where k <= q else MASKVAL
    # cols 128:384 -> shifted block: 0 where k <= q - 128 else MASKVAL
    kk = np.arange(128)[:, None]
    mt = np.where(kk <= np.arange(128)[None, :], 0.0, MASKVAL)
    md = np.where(kk <= np.arange(256)[None, :] - 128, 0.0, MASKVAL)
    mask = np.concatenate([mt, md], axis=1).astype(np.float32)

    consts = np.concatenate(
        [np.eye(128, dtype=np.float32),
         np.ones((128, 64), dtype=np.float32)], axis=1)
    import ml_dtypes
    conesb = np.ones((128, 64), dtype=ml_dtypes.bfloat16)

    in_maps = []
    for c in range(NCORES):
        Wc = np.empty((D, FPC), dtype=np.float32)
        bc = np.empty((FPC,), dtype=np.float32)
        for i, h in enumerate((HPC * c, HPC * c + 1)):
            Wc[:, 64 * i:64 * (i + 1)] = Wkqv[h][:, 0:64]          # k
            Wc[:, 128 + 64 * i:128 + 64 * (i + 1)] = \
                Wkqv[h][:, 64:128] / 8.0                            # q scaled
            Wc[:, 256 + 64 * i:256 + 64 * (i + 1)] = Wkqv[h][:, 128:192]  # v
            bc[64 * i:64 * (i + 1)] = bkqv[h][0:64]
            bc[128 + 64 * i:128 + 64 * (i + 1)] = bkqv[h][64:128] / 8.0
            bc[256 + 64 * i:256 + 64 * (i + 1)] = bkqv[h][128:192]
        in_maps.append({"xT": xT, "W": np.ascontiguousarray(Wc),
                        "bias": bc, "mask": mask, "consts": consts,
                        "conesb": conesb})
    return in_maps


def kernel(x, Wkqv, bkqv):
    global LAST_EXEC_NS
    _install_ntff_hook()
    from concourse.bass_utils import run_bass_kernel_spmd

    if "nc" not in _CACHE:
        _CACHE["nc"] = _build()
    nc = _CACHE["nc"]

    x = np.asarray(x, dtype=np.float32)
    Wkqv = np.asarray(Wkqv, dtype=np.float32)
    bkqv = np.asarray(bkqv, dtype=np.float32)
    in_maps = _host_prep(x, Wkqv, bkqv)

    trace = os.environ.get("BASS_KERNEL_TRACE", "0") == "1"
    res = run_bass_kernel_spmd(nc, in_maps, list(range(NCORES)), trace=trace)
    LAST_EXEC_NS = res.exec_time_ns

    out = np.empty((B, N, D), dtype=np.float32)
    for c in range(NCORES):
        out[:, :, 128 * c:128 * (c + 1)] = res.results[c]["out"]
    return out


# revision 31
# speedup vs baseline: 1.3696x; 1.0410x over previous
"""Causal self-attention Trainium2 kernel.

Problem: x [2, 2048, 1024], per-head stacked QKV weights Wkqv [16, 1024, 192]
(torch split order k,q,v), bias bkqv [16, 192]. Output [2, 2048, 1024]
(concat of per-head attention outputs, no output projection).

Sharding: tensor-parallel over heads — 8 cores x 2 heads each. No
collectives; host slices weights per core and concatenates outputs.

Per-core dataflow (all "transposed world": feature dim on partitions):
  - host passes xT [1024, 4096] (d-major; cols = batch0 seq ++ batch1 seq)
  - proj^T: psum[f_tile, n] = W[d, f_tile].T @ xT[d, n] accumulated over d
    -> kT/qT/vT [64*2heads, 2048] per batch (q pre-scaled by 1/8 on host)
  - scores^T tile: S^T[k_tile, q] = kT[:, k_tile].T @ qT[:, q_chunk]
  - additive causal mask on diagonal tiles, exp on ScalarE (no max
    subtraction: scores are O(1) bounded), result fp32r
  - out^T[65, q] += [V | 1].T @ expS^T accumulated over k tiles; row 64
    is the softmax denominator
  - PE-transpose out^T 128-col blocks -> [128, 65], divide by denom
    (per-partition scalar), DMA to out[b, q_block, head_cols]
"""

import os
import sys
import types

import numpy as np

B = 2
N = 2048
D = 1024
H = 16
HD = 64
NCORES = 8
HPC = H // NCORES  # heads per core = 2
FPC = 3 * HD * HPC  # 384 packed feature cols per core (k|k|q|q|v|v)
NCH = N // 512  # 4 q-chunks of 512 per batch
NKT = N // 128  # 16 k-tiles per batch
MASKVAL = -30000.0

LAST_EXEC_NS = None
_CACHE = {}


def _install_ntff_hook():
    try:
        import antenv.axon_hooks  # noqa: F401
        return
    except ImportError:
        pass
    try:
        import antenv
        from trn_agent_boot.trn_boot import _ntff_profile_via_ctypes
        hook = _ntff_profile_via_ctypes("/opt/axon/libaxon_pjrt.so")
        mod = types.ModuleType("antenv.axon_hooks")
        mod.get_axon_ntff_profile_hook = lambda: hook
        mod.set_axon_ntff_profile_hook = lambda h: None
        sys.modules["antenv.axon_hooks"] = mod
        antenv.axon_hooks = mod
    except Exception:
        pass


def _build():
    import concourse.bass as bass  # noqa: F401
    import concourse.mybir as mybir
    import concourse.tile as tile
    from concourse import bacc

    f32 = mybir.dt.float32
    f32r = mybir.dt.float32r
    f16 = mybir.dt.float16
    Exp = mybir.ActivationFunctionType.Exp

    nc = bacc.Bacc("TRN2", target_bir_lowering=False, debug=False,
                   num_devices=NCORES)

    xT_d = nc.dram_tensor("xT", [D, B * N], f32r, kind="ExternalInput").ap()
    W_d = nc.dram_tensor("W", [D, FPC], f32r, kind="ExternalInput").ap()
    b_d = nc.dram_tensor("bias", [FPC], f32, kind="ExternalInput").ap()
    m_d = nc.dram_tensor("mask", [128, 384], f32, kind="ExternalInput").ap()
    c_d = nc.dram_tensor("consts", [128, 192], f32r,
                         kind="ExternalInput").ap()
    ch_d = nc.dram_tensor("conesh", [128, 64], f16,
                          kind="ExternalInput").ap()
    o_d = nc.dram_tensor("out", [B, N, HPC * HD], f32,
                         kind="ExternalOutput").ap()

    with tile.TileContext(nc) as tc:
        with (
            tc.tile_pool(name="singles", bufs=1) as singles,
            tc.tile_pool(name="xt", bufs=16) as xt_pool,
            tc.tile_pool(name="kq", bufs=4) as kq_pool,
            tc.tile_pool(name="vt", bufs=1) as vt_pool,
            tc.tile_pool(name="vp", bufs=2) as vp_pool,
            tc.tile_pool(name="est", bufs=10) as est_pool,
            tc.tile_pool(name="ot", bufs=4) as ot_pool,
            tc.tile_pool(name="of", bufs=4) as of_pool,
            tc.tile_pool(name="ps_st", bufs=5, space="PSUM") as ps_st,
            tc.tile_pool(name="ps_ov", bufs=3, space="PSUM") as ps_ov,
        ):
            consts = singles.tile([128, 192], f32r)
            nc.gpsimd.dma_start(out=consts, in_=c_d)
            ident = consts[:, 0:128]
            mask_sb = singles.tile([128, 384], f32)
            nc.gpsimd.dma_start(out=mask_sb, in_=m_d)
            bias_sb = singles.tile([128, 3], f32)
            nc.gpsimd.dma_start(out=bias_sb,
                                in_=b_d.rearrange("(t p) -> p t", p=128))
            W_sb = singles.tile([128, 8, FPC], f32r)
            nc.sync.dma_start(out=W_sb,
                              in_=W_d.rearrange("(t p) f -> p t f", p=128))

            for b in range(B):
                # ---- load xT (per d-tile, per 512-col chunk) and project
                # kT/qT/vT [128, N] (2 heads stacked), chunk-outer so matmuls
                # start as soon as the first chunk of columns lands ----
                kT = kq_pool.tile([128, N], f16, tag="kq")
                qT = kq_pool.tile([128, N], f16, tag="kq")
                vT = vt_pool.tile([128, N], f32r, tag="vt")
                dest = [kT, qT, vT]
                xts = [[None] * 8 for _ in range(NCH)]

                def load_chunk(ch):
                    for d in range(8):
                        t = xt_pool.tile([128, 512], f32r, tag="xt",
                                         name=f"xt_{b}_{ch}_{d}")
                        nc.sync.dma_start(
                            out=t,
                            in_=xT_d[128 * d:128 * (d + 1),
                                     N * b + 512 * ch:N * b + 512 * (ch + 1)])
                        xts[ch][d] = t

                load_chunk(0)
                for ch in range(NCH):
                    for fi in range(3):
                        pj = ps_st.tile([128, 512], f32, tag="st")
                        for d in range(8):
                            nc.tensor.matmul(
                                pj,
                                W_sb[:, d, 128 * fi:128 * (fi + 1)],
                                xts[ch][d],
                                start=(d == 0), stop=(d == 7))
                        nc.vector.tensor_scalar_add(
                            out=dest[fi][:, 512 * ch:512 * (ch + 1)],
                            in0=pj, scalar1=bias_sb[:, fi:fi + 1])
                    if ch + 1 < NCH:
                        load_chunk(ch + 1)

                # ---- V natural layout [k, hd] with ones column ----
                vp_b = vp_pool.tile([128, HPC, NKT, 66], f16, tag="vp")
                nc.sync.dma_start(
                    out=vp_b[:, :, :, 64:66],
                    in_=ch_d.rearrange(
                        "p (h t o) -> p h t o", h=HPC, o=2))
                for h in range(HPC):
                    for t in range(NKT):
                        vtp = ps_ov.tile([128, 66], f32r, tag="ov")
                        nc.tensor.transpose(
                            out=vtp[:, 0:64],
                            in_=vT[64 * h:64 * (h + 1),
                                   128 * t:128 * (t + 1)],
                            identity=ident[64 * h:64 * (h + 1),
                                           64 * h:64 * (h + 1)])
                        nc.vector.tensor_copy(out=vp_b[:, h, t, 0:64],
                                              in_=vtp[:, 0:64])

                # ---- attention: both heads' chains interleaved per
                # q-chunk so PE always has independent work while the
                # scores->mask->exp chain of the other head is in flight ----
                def ov_copy(ov, h, ch):
                    otsb = ot_pool.tile([66, 512], f32r, tag="ot",
                                        name=f"otsb_{b}_{ch}_{h}")
                    nc.vector.tensor_copy(out=otsb, in_=ov)
                    return otsb

                def normalize_emit(otsb, h, ch):
                    for s in range(4):
                        ott = ps_ov.tile([128, 66], f32r, tag="ov")
                        nc.tensor.transpose(
                            out=ott,
                            in_=otsb[:, 128 * s:128 * (s + 1)],
                            identity=ident[0:66, 0:66])
                        rec = of_pool.tile([128, 1], f32, tag="rec")
                        nc.vector.reciprocal(out=rec, in_=ott[:, 64:65])
                        of = of_pool.tile([128, 64], f32, tag="of")
                        nc.vector.tensor_scalar_mul(
                            out=of, in0=ott[:, 0:64], scalar1=rec)
                        nc.gpsimd.dma_start(
                            out=o_d[b,
                                    512 * ch + 128 * s:
                                    512 * ch + 128 * (s + 1),
                                    64 * h:64 * (h + 1)],
                            in_=of)

                def params(t, ch):
                    r = t - 4 * ch
                    if r < 0:
                        c0 = 0
                    elif r <= 2:
                        c0 = 128 * r
                    else:
                        c0 = 256
                    return r, c0, 512 - c0

                pending = []
                for ch in range(NCH):
                    nkt = 4 * ch + 4
                    ovs = [ps_ov.tile([66, 512], f32, tag="ov",
                                      name=f"ov_{b}_{ch}_{i}")
                           for i in range(HPC)]
                    ests = [{} for _ in range(HPC)]

                    def stage_scores(h, t, ch=ch, ests=ests):
                        r, c0, w = params(t, ch)
                        hs = slice(64 * h, 64 * (h + 1))
                        st = ps_st.tile([128, 512], f32, tag="st")
                        nc.tensor.matmul(
                            st[:, 0:w],
                            kT[hs, 128 * t:128 * (t + 1)],
                            qT[hs, 512 * ch + c0:512 * (ch + 1)],
                            start=True, stop=True,
                            tile_position=(64 * h, 0))
                        if r >= 0:
                            if r <= 2:
                                nc.vector.tensor_add(
                                    out=st[:, 0:128], in0=st[:, 0:128],
                                    in1=mask_sb[:, 0:128])
                            else:
                                nc.vector.tensor_add(
                                    out=st[:, 0:256], in0=st[:, 0:256],
                                    in1=mask_sb[:, 128:384])
                        est = est_pool.tile([128, 512], f16, tag="est")
                        nc.scalar.activation(out=est[:, 0:w],
                                             in_=st[:, 0:w], func=Exp)
                        ests[h][t] = est

                    def stage_av(h, t, ch=ch, nkt=nkt, ests=ests, ovs=ovs):
                        r, c0, w = params(t, ch)
                        nc.tensor.matmul(
                            ovs[h][:, c0:512],
                            vp_b[:, h, t, :],
                            ests[h].pop(t)[:, 0:w],
                            start=(t == 0), stop=(t == nkt - 1))

                    LOOK = 2
                    for t in range(min(LOOK, nkt)):
                        for h in range(HPC):
                            stage_scores(h, t)
                    for otsb_p, h_p, ch_p in pending:
                        normalize_emit(otsb_p, h_p, ch_p)
                    pending = []
                    for t in range(nkt):
                        if t + LOOK < nkt:
                            for h in range(HPC):
                                stage_scores(h, t + LOOK)
                        for h in range(HPC):
                            stage_av(h, t)
                    for h in range(HPC):
                        pending.append((ov_copy(ovs[h], h, ch), h, ch))
                for otsb_p, h_p, ch_p in pending:
                    normalize_emit(otsb_p, h_p, ch_p)

    nc.compile()
    return nc


def _host_prep(x, Wkqv, bkqv):
    xT = np.ascontiguousarray(
        np.concatenate([x[0].T, x[1].T], axis=1)).astype(np.float32)

    # causal additive masks (k = partition/row, q = col), packed [128, 384]:
    # cols 0:128  -> triangle block: 0 where k <= q else MASKVAL
    # cols 128:384 -> shifted block: 0 where k <= q - 128 else MASKVAL
    kk = np.arange(128)[:, None]
    mt = np.where(kk <= np.arange(128)[None, :], 0.0, MASKVAL)
    md = np.where(kk <= np.arange(256)[None, :] - 128, 0.0, MASKVAL)
    mask = np.concatenate([mt, md], axis=1).astype(np.float32)

    consts = np.concatenate(
        [np.eye(128, dtype=np.float32),
         np.ones((128, 64), dtype=np.float32)], axis=1)
    conesh = np.ones((128, 64), dtype=np.float16)

    in_maps = []
    for c in range(NCORES):
        Wc = np.empty((D, FPC), dtype=np.float32)
        bc = np.empty((FPC,), dtype=np.float32)
        for i, h in enumerate((HPC * c, HPC * c + 1)):
            Wc[:, 64 * i:64 * (i + 1)] = Wkqv[h][:, 0:64]          # k
            Wc[:, 128 + 64 * i:128 + 64 * (i + 1)] = \
                Wkqv[h][:, 64:128] / 8.0                            # q scaled
            Wc[:, 256 + 64 * i:256 + 64 * (i + 1)] = Wkqv[h][:, 128:192]  # v
            bc[64 * i:64 * (i + 1)] = bkqv[h][0:64]
            bc[128 + 64 * i:128 + 64 * (i + 1)] = bkqv[h][64:128] / 8.0
            bc[256 + 64 * i:256 + 64 * (i + 1)] = bkqv[h][128:192]
        in_maps.append({"xT": xT, "W": np.ascontiguousarray(Wc),
                        "bias": bc, "mask": mask, "consts": consts,
                        "conesh": conesh})
    return in_maps


def kernel(x, Wkqv, bkqv):
    global LAST_EXEC_NS
    _install_ntff_hook()
    from concourse.bass_utils import run_bass_kernel_spmd

    if "nc" not in _CACHE:
        _CACHE["nc"] = _build()
    nc = _CACHE["nc"]

    x = np.asarray(x, dtype=np.float32)
    Wkqv = np.asarray(Wkqv, dtype=np.float32)
    bkqv = np.asarray(bkqv, dtype=np.float32)
    in_maps = _host_prep(x, Wkqv, bkqv)

    trace = os.environ.get("BASS_KERNEL_TRACE", "0") == "1"
    res = run_bass_kernel_spmd(nc, in_maps, list(range(NCORES)), trace=trace)
    LAST_EXEC_NS = res.exec_time_ns

    out = np.empty((B, N, D), dtype=np.float32)
    for c in range(NCORES):
        out[:, :, 128 * c:128 * (c + 1)] = res.results[c]["out"]
    return out
